# revision 5
# baseline (speedup 1.0000x reference)
"""Bass/Tile TRN2 kernel for nn_Actor_DeepSet (8-core data parallel), v2.

Per row r = b*8 + i (obs=64, hidden=128):
  h1   = relu(x_r @ w1.T + b1)
  hsum = (1/8) * sum_{a=1..7} relu(rot_{i+1}(x_{b,a}) @ w1o.T + b1o)
  h2   = relu([h1, hsum] @ w2.T + b2)
  out  = h2 @ wv.T + bv
rot_s is a feature rotation == column rotation of w1o; 1/8 folds into
w1o/b1o.

v2 layout: transposed (channels on partitions, rows on free axis), bf16.
x is [64, 16384] per core (K=64 matmuls, no zero padding).  Columns are
agent-major per 512-row tile (col j = a*64 + b).  Per pair (1024 rows):
  - 9 L1 stationaries (w1 + 8 shifts) cycle through a 3-buf PSUM pool;
    each shift's products drain (relu+bias) into a k-major slab buffer
    r[p, k=7, t=2, s=8, b=64]; the k-sum runs as 6 big tensor_tensor
    adds split across DVE and GpSimd (engine map knobs).
  - L2 = w2a@htop + w2b@hbot accumulated in PSUM, drained on Act.
  - L3 is packed transposed: 8 matmuls with h2-chunks stationary and
    wv moving -> ps3[128, 128]; out = copy(Act) + bvp add(GpSimd).
Three-stage software pipeline: front(g) | back2(g-2) | back3(g-3).
Host reorders input/output; all biases applied on device.
"""

import os
import time

import numpy as np

import concourse.bacc as bacc
import concourse.mybir as mybir
import concourse.tile as tile
from concourse.bass_utils import run_bass_kernel_spmd

N_CORES = 8
N_AGENTS = 8
OBS = 64
HIDDEN = 128
NUM_OUT = 16
ROWS_PC = 16384
TILE_N = 512
N_TILES = ROWS_PC // TILE_N
NB = TILE_N // N_AGENTS  # 64
PAIR_N = 2 * TILE_N      # 1024
N_PAIRS = N_TILES // 2   # 16

# --- tuning knobs ---
# engine for each of the 8 shift drains: V=vector, A=scalar(act)
KN_DRAIN = os.environ.get("KN_DRAIN", "VVVVAAAA")
# engine for the 6 k-sum tree ops [t1=r0+r1, t2=r2+r3, t3=r4+r5,
#   t1+=t2, t1+=t3, hbot=t1+r6]: V=vector, P=gpsimd
KN_TREE = os.environ.get("KN_TREE", "VPPVVV")
KN_HT = os.environ.get("KN_HT", "A")     # htop drain engine (A/V)
KN_H2 = os.environ.get("KN_H2", "A")     # h2 drain engine (A/V)
KN_OUT = os.environ.get("KN_OUT", "AP")  # out: copy engine + add engine
KN_FOLD = int(os.environ.get("KN_FOLD", "0"))   # k-slabs folded into L2
KN_PACKL3 = bool(int(os.environ.get("KN_PACKL3", "1")))

BF16 = mybir.dt.bfloat16
F32 = mybir.dt.float32
NP_BF16 = mybir.dt.np(BF16)
ALU = mybir.AluOpType
AF = mybir.ActivationFunctionType

_compiled_nc = None
last_exec_time_ns = None


def _build_nc():
    nc = bacc.Bacc("TRN2", target_bir_lowering=False, debug=False,
                   num_devices=N_CORES)

    x_ext = nc.dram_tensor("x", [OBS, ROWS_PC], BF16, kind="ExternalInput")
    wblob_ext = nc.dram_tensor("wblob", [HIDDEN, 1424], BF16,
                               kind="ExternalInput")
    bcat_ext = nc.dram_tensor("bcat", [HIDDEN, 4], F32, kind="ExternalInput")
    if KN_PACKL3:
        y_ext = nc.dram_tensor("y", [HIDDEN, N_PAIRS * HIDDEN], F32,
                               kind="ExternalOutput")
        bvp_ext = nc.dram_tensor("bvp", [HIDDEN, HIDDEN], F32,
                                 kind="ExternalInput")
    else:
        y_ext = nc.dram_tensor("y", [NUM_OUT, ROWS_PC], F32,
                               kind="ExternalOutput")

    def eng(c):
        return {"V": nc.vector, "A": nc.scalar, "P": nc.gpsimd}[c]

    with tile.TileContext(nc) as tc:
        with (
            tc.tile_pool(name="const", bufs=1) as cpool,
            tc.tile_pool(name="xin", bufs=4) as xpool,
            tc.tile_pool(name="rbuf", bufs=2) as rpool,
            tc.tile_pool(name="ttmp", bufs=2) as tpool,
            tc.tile_pool(name="htop", bufs=3) as htpool,
            tc.tile_pool(name="hbot", bufs=2) as hbpool,
            tc.tile_pool(name="h2b", bufs=2) as h2pool,
            tc.tile_pool(name="outb", bufs=3) as opool,
            tc.tile_pool(name="ps1", bufs=3, space="PSUM") as ps1pool,
            tc.tile_pool(name="ps2", bufs=1, space="PSUM") as ps2pool,
        ):
            wblob = cpool.tile([HIDDEN, 1424], BF16)
            nc.gpsimd.dma_start(wblob[:], wblob_ext[:])
            wl1 = wblob[:OBS, 0:HIDDEN]
            wsh = [wblob[:OBS, HIDDEN + s * HIDDEN:HIDDEN + (s + 1) * HIDDEN]
                   for s in range(N_AGENTS)]
            w2a = wblob[:, 1152:1280]
            w2b = wblob[:, 1280:1408]
            wv = wblob[:, 1408:1424]
            bcat = cpool.tile([HIDDEN, 4], F32)
            nc.gpsimd.dma_start(bcat[:], bcat_ext[:])
            b1t = bcat[:, 0:1]
            b1ot = bcat[:, 1:2]
            b2t = bcat[:, 2:3]
            bvt = bcat[:NUM_OUT, 3:4]
            if KN_PACKL3:
                bvp = cpool.tile([HIDDEN, HIDDEN], F32)
                nc.gpsimd.dma_start(bvp[:], bvp_ext[:])

            def drain(dst, src, bias, e):
                """relu(src + bias) -> dst (PSUM -> SBUF)."""
                if e == "A":
                    nc.scalar.activation(dst, src, AF.Relu, bias=bias)
                else:
                    nc.vector.tensor_scalar(dst, src, bias, 0.0,
                                            ALU.add, ALU.max)

            n_sum = 7 - KN_FOLD  # slabs summed on EW engines (rest folded)

            def front(g):
                col0 = g * PAIR_N
                xt = xpool.tile([OBS, PAIR_N], BF16)
                nc.sync.dma_start(xt[:], x_ext[:, col0:col0 + PAIR_N])

                # L1 main
                psm = ps1pool.tile([HIDDEN, PAIR_N], F32, tag="ps")
                nc.tensor.matmul(psm[:, :TILE_N], wl1, xt[:, :TILE_N])
                nc.tensor.matmul(psm[:, TILE_N:], wl1, xt[:, TILE_N:])
                htop = htpool.tile([HIDDEN, PAIR_N], BF16, tag="htop")
                drain(htop[:], psm[:], b1t, KN_HT)

                # L1 other: r layout [128, k=7, t=2, s=8, b=64]
                r = rpool.tile([HIDDEN, 7 * PAIR_N], BF16)
                r_v = r[:].rearrange("p (k t s b) -> p k t s b",
                                     k=7, t=2, s=N_AGENTS)
                for s in range(N_AGENTS):
                    ps = ps1pool.tile([HIDDEN, PAIR_N], F32, tag="ps")
                    for ti in range(2):
                        nc.tensor.matmul(
                            ps[:, ti * TILE_N:ti * TILE_N + 7 * NB],
                            wsh[s], xt[:, ti * TILE_N + NB:(ti + 1) * TILE_N])
                    src = ps[:].rearrange("p (t c) -> p t c", t=2)
                    src = src[:, :, :7 * NB].rearrange(
                        "p t (k b) -> p k t b", k=7)
                    drain(r_v[:, :, :, s, :], src, b1ot, KN_DRAIN[s])
                return r, htop

            def tree(g, state):
                """k-sum of r slabs 0..n_sum-1 -> hbot (6-KN_FOLD ops)."""
                r, htop = state
                r_k = r[:].rearrange("p (k c) -> p k c", k=7)
                hbot = hbpool.tile([HIDDEN, PAIR_N], BF16, tag="hbot")
                with nc.allow_low_precision("bf16 partial sums"):
                    if n_sum == 1:
                        eng(KN_TREE[0]).tensor_copy(hbot[:], r_k[:, 0, :])
                    elif n_sum <= 3:
                        eng(KN_TREE[0]).tensor_add(hbot[:], r_k[:, 0, :],
                                                   r_k[:, 1, :])
                        for k in range(2, n_sum):
                            eng(KN_TREE[k - 1]).tensor_add(
                                hbot[:], hbot[:], r_k[:, k, :])
                    else:
                        # t1(->hbot)=r0+r1, t2=r2+r3 [,t3=r4+r5],
                        # hbot+=t2 [,+=t3] [,+=r6]
                        tmp = tpool.tile([HIDDEN, 2 * PAIR_N], BF16)
                        t2 = tmp[:, :PAIR_N]
                        t3 = tmp[:, PAIR_N:]
                        eng(KN_TREE[0]).tensor_add(hbot[:], r_k[:, 0, :],
                                                   r_k[:, 1, :])
                        eng(KN_TREE[1]).tensor_add(t2[:], r_k[:, 2, :],
                                                   r_k[:, 3, :])
                        if n_sum >= 6:
                            eng(KN_TREE[2]).tensor_add(t3[:], r_k[:, 4, :],
                                                       r_k[:, 5, :])
                        eng(KN_TREE[3]).tensor_add(hbot[:], hbot[:], t2[:])
                        if n_sum >= 6:
                            eng(KN_TREE[4]).tensor_add(hbot[:], hbot[:],
                                                       t3[:])
                        if n_sum == 5:
                            eng(KN_TREE[4]).tensor_add(hbot[:], hbot[:],
                                                       r_k[:, 4, :])
                        if n_sum == 7:
                            eng(KN_TREE[5]).tensor_add(hbot[:], hbot[:],
                                                       r_k[:, 6, :])
                return r, htop, hbot

            def back2(g, state):
                r, htop, hbot = state
                r_kt = r[:].rearrange("p (k t c) -> p k t c", k=7, t=2)
                ps2 = ps2pool.tile([HIDDEN, PAIR_N], F32, tag="ps2")
                for ti in range(2):
                    sl = slice(ti * TILE_N, (ti + 1) * TILE_N)
                    nc.tensor.matmul(ps2[:, sl], w2a, htop[:, sl],
                                     start=True, stop=False)
                for ti in range(2):
                    sl = slice(ti * TILE_N, (ti + 1) * TILE_N)
                    nc.tensor.matmul(ps2[:, sl], w2b, hbot[:, sl],
                                     start=False, stop=(KN_FOLD == 0))
                    for k in range(n_sum, 7):
                        nc.tensor.matmul(ps2[:, sl], w2b, r_kt[:, k, ti, :],
                                         start=False, stop=(k == 6))
                h2 = h2pool.tile([HIDDEN, PAIR_N], BF16, tag="h2")
                drain(h2[:], ps2[:], b2t, KN_H2)
                return h2

            def back3_packed(g, h2):
                ps3 = ps2pool.tile([HIDDEN, HIDDEN], F32, tag="ps2",
                                   padded_shape=[HIDDEN, PAIR_N])
                for c in range(8):
                    nc.tensor.matmul(ps3[:, c * NUM_OUT:(c + 1) * NUM_OUT],
                                     h2[:, c * HIDDEN:(c + 1) * HIDDEN], wv)
                o = opool.tile([HIDDEN, HIDDEN], F32)
                if KN_OUT[0] == "A":
                    nc.scalar.copy(o[:], ps3[:])
                    with nc.allow_low_precision("bias add"):
                        eng(KN_OUT[1]).tensor_add(o[:], o[:], bvp[:])
                else:
                    eng(KN_OUT[0]).tensor_add(o[:], ps3[:], bvp[:])
                nc.sync.dma_start(
                    y_ext[:, g * HIDDEN:(g + 1) * HIDDEN], o[:])

            def back3_plain(g, h2):
                col0 = g * PAIR_N
                ps3 = ps2pool.tile([NUM_OUT, PAIR_N], F32, tag="ps2")
                for ti in range(2):
                    sl = slice(ti * TILE_N, (ti + 1) * TILE_N)
                    nc.tensor.matmul(ps3[:, sl], wv, h2[:, sl])
                o = opool.tile([NUM_OUT, PAIR_N], F32)
                if KN_OUT[0] == "A":
                    nc.scalar.activation(o[:], ps3[:], AF.Identity, bias=bvt)
                else:
                    nc.vector.tensor_scalar_add(o[:], ps3[:], bvt)
                nc.sync.dma_start(y_ext[:, col0:col0 + PAIR_N], o[:])

            back3 = back3_packed if KN_PACKL3 else back3_plain

            states = {}
            h2s = {}
            for g in range(N_PAIRS + 3):
                if g < N_PAIRS:
                    states[g] = front(g)
                if 1 <= g <= N_PAIRS:
                    states[g - 1] = tree(g - 1, states[g - 1])
                if g >= 3:
                    back3(g - 3, h2s.pop(g - 3))
                if 2 <= g <= N_PAIRS + 1:
                    h2s[g - 2] = back2(g - 2, states.pop(g - 2))

    nc.compile()
    return nc


def kernel(inputs, w1, b1, w1o, b1o, w2, b2, wv, bv):
    global _compiled_nc, last_exec_time_ns
    if _compiled_nc is None:
        _compiled_nc = _build_nc()
    nc = _compiled_nc

    inputs = np.asarray(inputs, dtype=np.float32)
    w1 = np.asarray(w1, dtype=np.float32)
    b1 = np.asarray(b1, dtype=np.float32)
    w1o = np.asarray(w1o, dtype=np.float32)
    b1o = np.asarray(b1o, dtype=np.float32)
    w2 = np.asarray(w2, dtype=np.float32)
    b2 = np.asarray(b2, dtype=np.float32)
    wv = np.asarray(wv, dtype=np.float32)
    bv = np.asarray(bv, dtype=np.float32)

    wblob = np.zeros((HIDDEN, 1424), dtype=NP_BF16)
    wblob[:OBS, 0:HIDDEN] = w1.T.astype(NP_BF16)
    for si in range(N_AGENTS):
        wblob[:OBS, HIDDEN + si * HIDDEN:HIDDEN + (si + 1) * HIDDEN] = \
            (np.roll(w1o, si + 1, axis=1).T / N_AGENTS).astype(NP_BF16)
    wblob[:, 1152:1280] = w2[:, :HIDDEN].T.astype(NP_BF16)
    wblob[:, 1280:1408] = w2[:, HIDDEN:].T.astype(NP_BF16)
    wblob[:, 1408:1424] = wv.T.astype(NP_BF16)
    bcat = np.zeros((HIDDEN, 4), dtype=np.float32)
    bcat[:, 0] = b1
    bcat[:, 1] = b1o / N_AGENTS
    bcat[:, 2] = b2
    bcat[:NUM_OUT, 3] = bv
    bvp = np.tile(bv.astype(np.float32), (HIDDEN, HIDDEN // NUM_OUT))

    # x columns agent-major per 512-row tile: col j = a*64 + b
    xs = inputs.reshape(N_CORES, N_TILES, NB, N_AGENTS, OBS)
    xs_t = np.ascontiguousarray(
        xs.transpose(0, 4, 1, 3, 2)).reshape(N_CORES, OBS, ROWS_PC)
    xs_t = xs_t.astype(NP_BF16)
    in_maps = []
    for c in range(N_CORES):
        m = {"x": xs_t[c], "wblob": wblob, "bcat": bcat}
        if KN_PACKL3:
            m["bvp"] = bvp
        in_maps.append(m)

    trace = bool(int(os.environ.get("BASS_KERNEL_TRACE", "0")))
    res = None
    for attempt in range(3):
        try:
            res = run_bass_kernel_spmd(nc, in_maps, list(range(N_CORES)),
                                       trace=trace)
            break
        except Exception:
            # transient NRT_EXEC_UNIT_UNRECOVERABLE happens ~5% of runs
            if attempt == 2:
                raise
            time.sleep(2.0)
    last_exec_time_ns = res.exec_time_ns

    y = np.stack([res.results[c]["y"] for c in range(N_CORES)])
    if KN_PACKL3:
        # y[p, pair*128 + c*16 + o]; h2-col j = c*128 + p = t*512+i*64+b
        y = y.reshape(N_CORES, HIDDEN, N_PAIRS, 8, NUM_OUT)
        y = y.transpose(0, 2, 3, 1, 4).reshape(
            N_CORES, N_PAIRS, PAIR_N, NUM_OUT)
        y = y.reshape(N_CORES, N_PAIRS, 2, N_AGENTS, NB, NUM_OUT)
        out = y.transpose(0, 1, 2, 4, 3, 5).reshape(
            N_CORES * ROWS_PC, NUM_OUT)
    else:
        y = y.reshape(N_CORES, NUM_OUT, N_PAIRS, 2, N_AGENTS, NB)
        out = y.transpose(0, 2, 3, 5, 4, 1).reshape(
            N_CORES * ROWS_PC, NUM_OUT)
    return np.ascontiguousarray(out, dtype=np.float32)


# revision 6
# speedup vs baseline: 1.1598x; 1.1598x over previous
"""Bass/Tile TRN2 kernel for nn_Actor_DeepSet (8-core data parallel), v3.

Per row r = b*8 + i (obs=64, hidden=128):
  h1   = relu(x_r @ w1.T + b1)
  hsum = (1/8) * sum_{a=1..7} relu(rot_{i+1}(x_{b,a}) @ w1o.T + b1o)
  h2   = relu([h1, hsum] @ w2.T + b2)
  out  = h2 @ wv.T + bv
rot_s is a feature rotation == column rotation of w1o; 1/8 folds into
w1o/b1o.

v3 = baseline dense schedule + K=64 L1 weights (no zero padding: halves
input DMA and LDWEIGHTS time for 18 of 24 matmuls/pair).  The PE p-state
ramps only under continuous load, so the schedule keeps PE dense: one
shared 4-buf PSUM pool, shallow front/back2/back3 pipeline.  Columns are
agent-major per 512-row tile (col j = a*64 + b).  Shift products drain
(relu+bias) into a k-major slab r[p,k=7,t=2,s=8,b=64]; the k-sum runs as
TT adds (engine per op via KN_TREE, V=vector P=gpsimd) with KN_FOLD
slabs folded into the L2 PSUM accumulation instead.  The final [16,1024]
out-drain is split half/half across Act and DVE.
"""

import os
import time

import numpy as np

import concourse.bacc as bacc
import concourse.mybir as mybir
import concourse.tile as tile
from concourse.bass_utils import run_bass_kernel_spmd

N_CORES = 8
N_AGENTS = 8
OBS = 64
HIDDEN = 128
NUM_OUT = 16
ROWS_PC = 16384
TILE_N = 512
N_TILES = ROWS_PC // TILE_N
NB = TILE_N // N_AGENTS  # 64
PAIR_N = 2 * TILE_N      # 1024
N_PAIRS = N_TILES // 2   # 16

# --- tuning knobs ---
# shift-drain engine maps (V=vector, A=scalar), alternating by pair parity
KN_DRAIN0 = os.environ.get("KN_DRAIN0", "VVVAAAAA")
KN_DRAIN1 = os.environ.get("KN_DRAIN1", "VVVVAAAA")
# engines for the k-sum adds [t1=r0+r1, t2=r2+r3, t3=r4+r5, t1+=t2,
#   t1+=t3, (t1+=r6 if n_sum==7)]: V=vector, P=gpsimd
KN_TREE = os.environ.get("KN_TREE", "VPPVVV")
KN_HT = os.environ.get("KN_HT", "A")     # htop drain engine (A/V)
KN_H2 = os.environ.get("KN_H2", "A")     # h2 drain engine (A/V)
KN_OUT = os.environ.get("KN_OUT", "S")   # out drain: A, V, or S(plit)
KN_FOLD = int(os.environ.get("KN_FOLD", "1"))   # k-slabs folded into L2

BF16 = mybir.dt.bfloat16
F32 = mybir.dt.float32
NP_BF16 = mybir.dt.np(BF16)
ALU = mybir.AluOpType
AF = mybir.ActivationFunctionType

_compiled_nc = None
last_exec_time_ns = None


def _build_nc():
    nc = bacc.Bacc("TRN2", target_bir_lowering=False, debug=False,
                   num_devices=N_CORES)

    x_ext = nc.dram_tensor("x", [OBS, ROWS_PC], BF16, kind="ExternalInput")
    wblob_ext = nc.dram_tensor("wblob", [HIDDEN, 1424], BF16,
                               kind="ExternalInput")
    bcat_ext = nc.dram_tensor("bcat", [HIDDEN, 4], F32, kind="ExternalInput")
    y_ext = nc.dram_tensor("y", [NUM_OUT, ROWS_PC], F32,
                           kind="ExternalOutput")

    def eng(c):
        return {"V": nc.vector, "A": nc.scalar, "P": nc.gpsimd}[c]

    n_sum = 7 - KN_FOLD

    with tile.TileContext(nc) as tc:
        with (
            tc.tile_pool(name="const", bufs=1) as cpool,
            tc.tile_pool(name="xin", bufs=4) as xpool,
            tc.tile_pool(name="rbuf", bufs=2) as rpool,
            tc.tile_pool(name="ttmp", bufs=2) as tpool,
            tc.tile_pool(name="act", bufs=4) as apool,
            tc.tile_pool(name="outb", bufs=3) as opool,
            tc.tile_pool(name="ps", bufs=4, space="PSUM") as pps,
        ):
            wblob = cpool.tile([HIDDEN, 1424], BF16)
            nc.gpsimd.dma_start(wblob[:], wblob_ext[:])
            wl1 = wblob[:OBS, 0:HIDDEN]
            wsh = [wblob[:OBS, HIDDEN + s * HIDDEN:HIDDEN + (s + 1) * HIDDEN]
                   for s in range(N_AGENTS)]
            w2a = wblob[:, 1152:1280]
            w2b = wblob[:, 1280:1408]
            wv = wblob[:, 1408:1424]
            bcat = cpool.tile([HIDDEN, 4], F32)
            nc.gpsimd.dma_start(bcat[:], bcat_ext[:])
            b1t = bcat[:, 0:1]
            b1ot = bcat[:, 1:2]
            b2t = bcat[:, 2:3]
            bvt = bcat[:NUM_OUT, 3:4]

            def drain(dst, src, bias, e):
                """relu(src + bias) -> dst (PSUM -> SBUF)."""
                if e == "A":
                    nc.scalar.activation(dst, src, AF.Relu, bias=bias)
                else:
                    nc.vector.tensor_scalar(dst, src, bias, 0.0,
                                            ALU.add, ALU.max)

            def front(g):
                col0 = g * PAIR_N
                dmap = KN_DRAIN0 if g % 2 == 0 else KN_DRAIN1
                xt = xpool.tile([OBS, PAIR_N], BF16)
                nc.sync.dma_start(xt[:], x_ext[:, col0:col0 + PAIR_N])

                psm = pps.tile([HIDDEN, PAIR_N], F32, tag="ps")
                nc.tensor.matmul(psm[:, :TILE_N], wl1, xt[:, :TILE_N])
                nc.tensor.matmul(psm[:, TILE_N:], wl1, xt[:, TILE_N:])
                htop = apool.tile([HIDDEN, PAIR_N], BF16, tag="htop")
                drain(htop[:], psm[:], b1t, KN_HT)

                # r layout [128, k=7, t=2, s=8, b=64]
                r = rpool.tile([HIDDEN, 7 * PAIR_N], BF16)
                r_v = r[:].rearrange("p (k t s b) -> p k t s b",
                                     k=7, t=2, s=N_AGENTS)
                for s in range(N_AGENTS):
                    ps = pps.tile([HIDDEN, PAIR_N], F32, tag="ps")
                    for ti in range(2):
                        nc.tensor.matmul(
                            ps[:, ti * TILE_N:ti * TILE_N + 7 * NB],
                            wsh[s], xt[:, ti * TILE_N + NB:(ti + 1) * TILE_N])
                    src = ps[:].rearrange("p (t c) -> p t c", t=2)
                    src = src[:, :, :7 * NB].rearrange(
                        "p t (k b) -> p k t b", k=7)
                    drain(r_v[:, :, :, s, :], src, b1ot, dmap[s])

                # k-sum of slabs 0..n_sum-1 -> hbot
                r_k = r[:].rearrange("p (k c) -> p k c", k=7)
                hbot = apool.tile([HIDDEN, PAIR_N], BF16, tag="hbot")
                with nc.allow_low_precision("bf16 partial sums"):
                    if n_sum >= 6:
                        tmp = tpool.tile([HIDDEN, 2 * PAIR_N], BF16)
                        t2 = tmp[:, :PAIR_N]
                        t3 = tmp[:, PAIR_N:]
                        eng(KN_TREE[0]).tensor_add(hbot[:], r_k[:, 0, :],
                                                   r_k[:, 1, :])
                        eng(KN_TREE[1]).tensor_add(t2[:], r_k[:, 2, :],
                                                   r_k[:, 3, :])
                        eng(KN_TREE[2]).tensor_add(t3[:], r_k[:, 4, :],
                                                   r_k[:, 5, :])
                        eng(KN_TREE[3]).tensor_add(hbot[:], hbot[:], t2[:])
                        eng(KN_TREE[4]).tensor_add(hbot[:], hbot[:], t3[:])
                        if n_sum == 7:
                            eng(KN_TREE[5]).tensor_add(hbot[:], hbot[:],
                                                       r_k[:, 6, :])
                    else:
                        eng(KN_TREE[0]).tensor_add(hbot[:], r_k[:, 0, :],
                                                   r_k[:, 1, :])
                        for k in range(2, n_sum):
                            eng(KN_TREE[k - 1]).tensor_add(
                                hbot[:], hbot[:], r_k[:, k, :])
                return r, htop, hbot

            def back2(g, state):
                r, htop, hbot = state
                r_kt = r[:].rearrange("p (k t c) -> p k t c", k=7, t=2)
                ps2 = pps.tile([HIDDEN, PAIR_N], F32, tag="ps")
                for ti in range(2):
                    sl = slice(ti * TILE_N, (ti + 1) * TILE_N)
                    nc.tensor.matmul(ps2[:, sl], w2a, htop[:, sl],
                                     start=True, stop=False)
                for ti in range(2):
                    sl = slice(ti * TILE_N, (ti + 1) * TILE_N)
                    nc.tensor.matmul(ps2[:, sl], w2b, hbot[:, sl],
                                     start=False, stop=(KN_FOLD == 0))
                    for k in range(n_sum, 7):
                        nc.tensor.matmul(ps2[:, sl], w2b, r_kt[:, k, ti, :],
                                         start=False, stop=(k == 6))
                h2 = apool.tile([HIDDEN, PAIR_N], BF16, tag="h2")
                drain(h2[:], ps2[:], b2t, KN_H2)
                return h2

            def back3(g, h2):
                col0 = g * PAIR_N
                ps3 = pps.tile([NUM_OUT, PAIR_N], F32, tag="ps")
                for ti in range(2):
                    sl = slice(ti * TILE_N, (ti + 1) * TILE_N)
                    nc.tensor.matmul(ps3[:, sl], wv, h2[:, sl])
                o = opool.tile([NUM_OUT, PAIR_N], F32)
                if KN_OUT == "A":
                    nc.scalar.activation(o[:], ps3[:], AF.Identity, bias=bvt)
                elif KN_OUT == "V":
                    nc.vector.tensor_scalar_add(o[:], ps3[:], bvt)
                else:  # split across both engines
                    nc.scalar.activation(o[:, :TILE_N], ps3[:, :TILE_N],
                                         AF.Identity, bias=bvt)
                    nc.vector.tensor_scalar_add(o[:, TILE_N:],
                                                ps3[:, TILE_N:], bvt)
                nc.sync.dma_start(y_ext[:, col0:col0 + PAIR_N], o[:])

            states = {}
            h2s = {}
            for g in range(N_PAIRS + 2):
                if g < N_PAIRS:
                    states[g] = front(g)
                if 1 <= g <= N_PAIRS:
                    h2s[g - 1] = back2(g - 1, states.pop(g - 1))
                if g >= 2:
                    back3(g - 2, h2s.pop(g - 2))

    nc.compile()
    return nc


def kernel(inputs, w1, b1, w1o, b1o, w2, b2, wv, bv):
    global _compiled_nc, last_exec_time_ns
    if _compiled_nc is None:
        _compiled_nc = _build_nc()
    nc = _compiled_nc

    inputs = np.asarray(inputs, dtype=np.float32)
    w1 = np.asarray(w1, dtype=np.float32)
    b1 = np.asarray(b1, dtype=np.float32)
    w1o = np.asarray(w1o, dtype=np.float32)
    b1o = np.asarray(b1o, dtype=np.float32)
    w2 = np.asarray(w2, dtype=np.float32)
    b2 = np.asarray(b2, dtype=np.float32)
    wv = np.asarray(wv, dtype=np.float32)
    bv = np.asarray(bv, dtype=np.float32)

    wblob = np.zeros((HIDDEN, 1424), dtype=NP_BF16)
    wblob[:OBS, 0:HIDDEN] = w1.T.astype(NP_BF16)
    for si in range(N_AGENTS):
        wblob[:OBS, HIDDEN + si * HIDDEN:HIDDEN + (si + 1) * HIDDEN] = \
            (np.roll(w1o, si + 1, axis=1).T / N_AGENTS).astype(NP_BF16)
    wblob[:, 1152:1280] = w2[:, :HIDDEN].T.astype(NP_BF16)
    wblob[:, 1280:1408] = w2[:, HIDDEN:].T.astype(NP_BF16)
    wblob[:, 1408:1424] = wv.T.astype(NP_BF16)
    bcat = np.zeros((HIDDEN, 4), dtype=np.float32)
    bcat[:, 0] = b1
    bcat[:, 1] = b1o / N_AGENTS
    bcat[:, 2] = b2
    bcat[:NUM_OUT, 3] = bv

    # x columns agent-major per 512-row tile: col j = a*64 + b
    xs = inputs.reshape(N_CORES, N_TILES, NB, N_AGENTS, OBS)
    xs_t = np.ascontiguousarray(
        xs.transpose(0, 4, 1, 3, 2)).reshape(N_CORES, OBS, ROWS_PC)
    xs_t = xs_t.astype(NP_BF16)
    in_maps = [{"x": xs_t[c], "wblob": wblob, "bcat": bcat}
               for c in range(N_CORES)]

    trace = bool(int(os.environ.get("BASS_KERNEL_TRACE", "0")))
    res = None
    for attempt in range(3):
        try:
            res = run_bass_kernel_spmd(nc, in_maps, list(range(N_CORES)),
                                       trace=trace)
            break
        except Exception:
            # transient NRT_EXEC_UNIT_UNRECOVERABLE happens ~5% of runs
            if attempt == 2:
                raise
            time.sleep(2.0)
    last_exec_time_ns = res.exec_time_ns

    y = np.stack([res.results[c]["y"] for c in range(N_CORES)])
    y = y.reshape(N_CORES, NUM_OUT, N_PAIRS, 2, N_AGENTS, NB)
    out = y.transpose(0, 2, 3, 5, 4, 1).reshape(N_CORES * ROWS_PC, NUM_OUT)
    return np.ascontiguousarray(out, dtype=np.float32)
